# revision 84
# baseline (speedup 1.0000x reference)
"""Multi-layer bidirectional Tree-LSTM on 8 TRN2 NeuronCores.

Strategy: the input is a complete binary tree of 1024 nodes. Below level 3
there are 8 independent subtrees (rooted at nodes 7..14) -> one subtree per
core (data parallel). The top 7 nodes (0..6) are computed replicated on all
cores; one small AllGather per layer exchanges the 8 subtree-root (h, c)
pairs for the leaves->root direction.

On-device layout is feature-major (hidden dim on partitions, nodes on the
free axis); weights are stationary and node columns stream, so no
transposes are needed anywhere in the recurrence.

Per-core column layout (143 columns):
  0..126   : BFS slots of subtree(7+c)  (slot s, level k=floor(log2(s+1)))
  127      : node 1023 (replicated on every core; only core 0's is used)
  128..134 : top nodes 0..6 (replicated)
  135..142 : subtree roots 7..14 (fwd: from AllGather; bwd: replicated)

Precision: pre-projection weights bf16; recurrence weights fp8 e4m3
scaled by 64 (the recurrence is LDWEIGHTS-bandwidth-bound and fp8 FWL
loads 2x faster than bf16; the 1/64 un-scale is folded into the bf16
child/parent-h copies, which is a lossless exponent shift). Measured
end-to-end rel err ~6e-3 vs the 2e-2 gate.

Gate row order is permuted at pack time to pipeline the gate activations
into the level matmul phase: fwd [u ig fl fr og r] (tanh 0:4, sigmoid
4:24), bwd [u ig f og r] (tanh 0:4, sigmoid 4:20). u comes first so
tanh(u) fires after the first 4 m-chunks accumulate; og/r last (their
consumers sit at the end of the elementwise chain).

All weight DRAM tensors are host-packed to the exact SBUF layout so every
weight DMA is a plain contiguous [128, X] copy (large descriptors, full
HBM bandwidth). The layer pipeline enqueues layer l+1's full weight
stream on the SP DMA ring during layer l's pre-projections; all
collective-path DMAs (ccin write, gather read) and small constants ride
the Activation-engine DMA ring so they never queue behind the weight
stream. The AllGather input/output transposes run on the PE against a
staged identity (one contiguous DMA each way) instead of the old
per-element transposing DMAs. The bwd tail is emitted interleaved with
the fwd small levels and the post-AllGather fwd top levels, so each
chain's matmul bursts fill the other chain's elementwise latency and
the AllGather window stays full.
"""

import os
import sys

for _p in ("/opt/trn_rl_repo",):
    if _p not in sys.path and os.path.isdir(_p):
        sys.path.insert(0, _p)

import numpy as np
import ml_dtypes

try:
    import jax
    jax.config.update("jax_compilation_cache_dir", os.environ.get("KERNEL_JAX_CACHE", "/tmp/jax_neff_cache"))
    jax.config.update("jax_persistent_cache_min_compile_time_secs", 5.0)
    jax.config.update("jax_persistent_cache_min_entry_size_bytes", 0)
except Exception:
    pass

import concourse.bass as bass
import concourse.mybir as mybir
from concourse import bacc
from concourse.tile import TileContext
from concourse.bass_utils import run_bass_kernel_spmd

BF16 = ml_dtypes.bfloat16
FP8 = ml_dtypes.float8_e4m3fn
F32 = mybir.dt.float32
B16 = mybir.dt.bfloat16
F8 = mybir.dt.float8e4
AF = mybir.ActivationFunctionType

N, D, H, L = 1024, 1024, 512, 2
NCOL = 143  # 127 subtree + node1023 + 7 top + 8 roots
NCORES = 8
# fp8 recurrence scales: weights x64, h operand x4. The product (x256) is
# folded into the pre-projection gate rows/biases on the host, and undone
# by the activation `scale` parameter (powers of two -> exact).
WSCALE = 64.0
HSCALE = 4.0
GSCALE = WSCALE * HSCALE
GINV = 1.0 / GSCALE
# pre-projection group streaming order: PRE_F groups (0-6) first so the
# fwd chain (the long chain: leaves -> root -> AllGather -> top levels)
# can start as soon as its half of PRE is done.
GORDER = [0, 1, 2, 3, 4, 5, 6, 7, 8, 9, 10, 11, 12]

_last_results = None  # stashed BassKernelResults for test.py


def _node_ids(c):
    ids = []
    for k in range(7):
        base = (8 + c) * (1 << k) - 1
        ids.extend(range(base, base + (1 << k)))
    ids.append(1023)
    ids.extend(range(0, 7))
    ids.extend(range(7, 15))
    return np.asarray(ids, dtype=np.int64)


def _pack_lhsT(w, kchunks, mchunks, dty):
    # w: [M, K] fp32 -> lhsT tiles [kchunks, mchunks, 128, 128] where
    # tile[k, m, kp, mc] = w[m*128+mc, k*128+kp]
    Mdim, Kdim = w.shape
    assert Mdim == mchunks * 128 and Kdim == kchunks * 128
    t = w.reshape(mchunks, 128, kchunks, 128).transpose(2, 0, 3, 1)
    return np.ascontiguousarray(t.astype(dty))


def _perm_f(w):
    # fwd gate rows [ig og fl fr u r] -> [u ig fl fr og r]: u first so
    # tanh(u) fires after 4 m-chunks of the level matmul; og/r last (their
    # consumers are at the end of the elementwise chain anyway)
    return np.concatenate([w[2048:2560], w[0:512], w[1024:1536],
                           w[1536:2048], w[512:1024], w[2560:3072]], axis=0)


def _perm_b(w):
    # bwd gate rows [ig og f u r] -> [u ig f og r]
    return np.concatenate([w[1536:2048], w[0:512], w[1024:1536],
                           w[512:1024], w[2048:2560]], axis=0)


def _build_program():
    nc = bacc.Bacc("TRN2", target_bir_lowering=False, debug=False,
                   num_devices=NCORES)

    featsT_d = nc.dram_tensor("featsT", [128, 8 * NCOL], B16, kind="ExternalInput")
    wpre_d, wrecf_d, wrecb_d, biasf_d, biasb_d = [], [], [], [], []
    for l in range(L):
        wpre_d.append(nc.dram_tensor(f"wpre{l}", [13, 128, 4096], B16,
                                     kind="ExternalInput"))
        wrecf_d.append(nc.dram_tensor(f"wrecf{l}", [128, 8 * 24 * 128], F8,
                                      kind="ExternalInput"))
        wrecb_d.append(nc.dram_tensor(f"wrecb{l}", [128, 4 * 20 * 128], F8,
                                      kind="ExternalInput"))
        biasf_d.append(nc.dram_tensor(f"biasf{l}", [128, 28], F32,
                                      kind="ExternalInput"))
        biasb_d.append(nc.dram_tensor(f"biasb{l}", [128, 24], F32,
                                      kind="ExternalInput"))
    mask_d = nc.dram_tensor("mask", [128, 1], F32, kind="ExternalInput")
    psel_d = nc.dram_tensor("psel", [128, 8], F32, kind="ExternalInput")
    ident_d = nc.dram_tensor("ident", [128, 128], F32, kind="ExternalInput")
    out_loc_d = nc.dram_tensor("out_loc", [1024, 128], F32, kind="ExternalOutput")
    out_top_d = nc.dram_tensor("out_top", [1024, 7], F32, kind="ExternalOutput")

    with TileContext(nc) as tc:
        with (
            tc.tile_pool(name="state", bufs=1) as state_p,
            tc.tile_pool(name="weights", bufs=2) as w_p,
            # wb is single-buffered: layer l+1's wb DMA waits for layer l's
            # last bwd-step reader (~recurrence end) and lands well before
            # layer l+1's first bwd gemm. The freed SBUF pays for a deeper
            # wpre-group rotation (fewer trickling groups in layer 1's pre).
            tc.tile_pool(name="weights_b", bufs=1) as wb_p,
            tc.tile_pool(name="pre", bufs=1) as pre_p,
            tc.tile_pool(name="wstream", bufs=10) as ws_p,
            tc.tile_pool(name="scratch", bufs=2) as sc_p,
            tc.tile_pool(name="ccstage", bufs=1) as cc_p,
            tc.tile_pool(name="psum_pre", bufs=2, space="PSUM") as pp_p,
            tc.tile_pool(name="psum_rec", bufs=2, space="PSUM") as pr_p,
            tc.tile_pool(name="dram", bufs=1, space="DRAM") as dram_p,
        ):
            # HCF holds fwd h (chunks 0:4) and c (chunks 4:8) in one tile so
            # the collective input (h,c of the subtree root) is a single
            # [128, 8] slice -> one PE transpose -> one contiguous DMA.
            HCF = state_p.tile([128, 8, NCOL], F32, name="HCF")
            HB = state_p.tile([128, 4, NCOL], F32, name="HB")
            CB = state_p.tile([128, 4, NCOL], F32, name="CB")
            mask_sb = state_p.tile([128, 1], F32, name="mask_sb")
            psel_sb = state_p.tile([128, 8], F32, name="psel_sb")
            ident_sb = state_p.tile([128, 128], F32, name="ident_sb")

            def startup_extras():
                """Emitted after layer 0's critical weight DMAs so these
                don't occupy SP-ring slots ahead of the first matmul's
                inputs (small constants ride the Act ring). Includes the
                warmup collective: the first AllGather of a run costs ~3-5x
                the steady-state latency, so absorb it during initial weight
                streaming (CC cores are idle there)."""
                nc.scalar.dma_start(mask_sb[:], mask_d[:])
                nc.scalar.dma_start(psel_sb[:], psel_d[:])
                nc.scalar.dma_start(ident_sb[:], ident_d[:])
                ccw_in = dram_p.tile([8], F32, tag="ccwi", name="ccw_in")
                ccw_out = dram_p.tile([8, 8], F32, tag="ccwo", name="ccw_out",
                                      addr_space="Shared")
                nc.scalar.dma_start(ccw_in[0:8], psel_d[0, 0:8])
                nc.gpsimd.collective_compute(
                    "AllGather", mybir.AluOpType.bypass,
                    ins=[ccw_in.opt()], outs=[ccw_out.opt()],
                    replica_groups=[list(range(NCORES))])

            # current-layer tile handles (set by enqueue_weights)
            cur = {}

            def enqueue_weights(l, first):
                """Allocate layer-l weight tiles and enqueue all their DMAs
                on the SP ring. Order within the layer: biases, (feats),
                wpre groups 0-4, recurrence weights, wpre groups 5-12."""
                st = {}
                st["bf"] = w_p.tile([128, 28], F32, tag="bf", name="bf_sb")
                st["bb"] = w_p.tile([128, 24], F32, tag="bb", name="bb_sb")
                # Act ring: keeps these small descriptor-heavy copies out of
                # the SP ring ahead of the first weight group
                nc.scalar.dma_start(st["bf"][:], biasf_d[l][:])
                nc.scalar.dma_start(st["bb"][:], biasb_d[l][:])
                st["ft"] = pre_p.tile([128, 8, NCOL], B16, tag="ft", name="ftile")
                if first:
                    # Act ring: idle at startup, overlaps the SP weight stream
                    nc.scalar.dma_start(
                        st["ft"][:].rearrange("p k c -> p (k c)"), featsT_d[:])
                st["wp"] = {}
                st["wf"] = w_p.tile([128, 8 * 24 * 128], F8, tag="wf", name="wf_sb")
                st["wb"] = wb_p.tile([128, 4 * 20 * 128], F8, tag="wb", name="wb_sb")
                # PRE_B groups (7..12) stream first: their consumer (the bwd
                # root step) is early in the interleaved recurrence, so the
                # scheduler keeps their matmuls early and the next layer's
                # group DMAs (buffer-WAR on these readers) can prefetch.
                for i, gidx in enumerate(GORDER):
                    wpb = ws_p.tile([128, 8 * 4 * 128], B16, tag="wpre", name="wpb")
                    nc.sync.dma_start(wpb[:], wpre_d[l][gidx])
                    st["wp"][gidx] = wpb
                    if i == 8:
                        nc.sync.dma_start(st["wf"][:], wrecf_d[l][:])
                        nc.sync.dma_start(st["wb"][:], wrecb_d[l][:])
                return st

            def fwd_elem(lo, n, ps, lc, rc, do_mm=None):
                """gates -> (c, hf) for fwd columns [lo, lo+n).
                gate order: u ig fl fr og r  (tanh 0:4, sigmoid 4:24).
                Gate pre-activations are carried x GSCALE; the activation
                scale undoes it exactly. When do_mm is given, the level's
                matmuls are EMITTED in three m-chunk pieces interleaved
                with the gate activations, so each activation piece fires
                while later m-chunks are still accumulating and most of
                the elementwise chain hides under the matmul phase."""
                g = sc_p.tile([128, 24, 65], F32, tag="gates", name="g")
                cnew = HCF[:, 4:8, lo:lo + n]
                t1 = sc_p.tile([128, 4, 65], F32, tag="t1", name="t1")
                t2 = sc_p.tile([128, 4, 65], F32, tag="t2", name="t2")
                if ps is None:
                    nc.scalar.activation(g[:, 0:4, :n], PRE_F[:, 0:4, lo:lo + n],
                                         AF.Tanh, scale=GINV)
                    nc.scalar.activation(g[:, 4:24, :n], PRE_F[:, 4:24, lo:lo + n],
                                         AF.Sigmoid, scale=GINV)
                else:
                    do_mm(0, 4)
                    nc.vector.tensor_add(g[:, 0:4, :n], ps[:, 0:4, :n],
                                         PRE_F[:, 0:4, lo:lo + n])
                    nc.scalar.activation(g[:, 0:4, :n], g[:, 0:4, :n], AF.Tanh,
                                         scale=GINV)
                    do_mm(4, 16)
                    nc.vector.tensor_add(g[:, 4:16, :n], ps[:, 4:16, :n],
                                         PRE_F[:, 4:16, lo:lo + n])
                    nc.scalar.activation(g[:, 4:16, :n], g[:, 4:16, :n],
                                         AF.Sigmoid, scale=GINV)
                # c = ig*u (+ fl*lc + fr*rc)
                nc.vector.tensor_mul(cnew, g[:, 4:8, :n], g[:, 0:4, :n])
                if lc is not None:
                    # fr*rc runs on the (otherwise idle) Pool engine in
                    # parallel with fl*lc on DVE
                    nc.gpsimd.tensor_mul(t2[:, :, :n], g[:, 12:16, :n], rc)
                    nc.vector.tensor_mul(t1[:, :, :n], g[:, 8:12, :n], lc)
                    nc.vector.tensor_add(cnew, cnew, t1[:, :, :n])
                    nc.vector.tensor_add(cnew, cnew, t2[:, :, :n])
                nc.scalar.activation(t1[:, :, :n], cnew, AF.Tanh)
                if ps is not None:
                    do_mm(16, 24)
                    nc.vector.tensor_add(g[:, 16:24, :n], ps[:, 16:24, :n],
                                         PRE_F[:, 16:24, lo:lo + n])
                    nc.scalar.activation(g[:, 16:24, :n], g[:, 16:24, :n],
                                         AF.Sigmoid, scale=GINV)
                # hf = og*tanh(c)*r + (1-r)*px = r*(hh - px) + px
                px = PRE_F[:, 24:28, lo:lo + n]
                nc.vector.tensor_mul(t2[:, :, :n], g[:, 16:20, :n],
                                     t1[:, :, :n])
                nc.vector.tensor_sub(t2[:, :, :n], t2[:, :, :n], px)
                nc.vector.tensor_mul(t2[:, :, :n], g[:, 20:24, :n],
                                     t2[:, :, :n])
                nc.vector.tensor_add(HCF[:, 0:4, lo:lo + n], t2[:, :, :n], px)

            def bwd_elem(lo, n, ps, pc, do_mm=None):
                # gate order: u ig f og r  (tanh 0:4, sigmoid 4:20), split
                # into pieces matching the matmul m-chunk order
                g = sc_p.tile([128, 24, 65], F32, tag="gates", name="gb")
                cnew = CB[:, :, lo:lo + n]
                t1 = sc_p.tile([128, 4, 65], F32, tag="t1", name="t1b")
                t2 = sc_p.tile([128, 4, 65], F32, tag="t2", name="t2b")
                if ps is None:
                    nc.scalar.activation(g[:, 0:4, :n], PRE_B[:, 0:4, lo:lo + n],
                                         AF.Tanh, scale=GINV)
                    nc.scalar.activation(g[:, 4:20, :n], PRE_B[:, 4:20, lo:lo + n],
                                         AF.Sigmoid, scale=GINV)
                else:
                    do_mm(0, 4)
                    nc.vector.tensor_add(g[:, 0:4, :n], ps[:, 0:4, :n],
                                         PRE_B[:, 0:4, lo:lo + n])
                    nc.scalar.activation(g[:, 0:4, :n], g[:, 0:4, :n], AF.Tanh,
                                         scale=GINV)
                    do_mm(4, 12)
                    nc.vector.tensor_add(g[:, 4:12, :n], ps[:, 4:12, :n],
                                         PRE_B[:, 4:12, lo:lo + n])
                    nc.scalar.activation(g[:, 4:12, :n], g[:, 4:12, :n],
                                         AF.Sigmoid, scale=GINV)
                if pc is not None:
                    nc.gpsimd.tensor_mul(t1[:, :, :n], g[:, 8:12, :n], pc)
                nc.vector.tensor_mul(cnew, g[:, 4:8, :n], g[:, 0:4, :n])  # ig*u
                if pc is not None:
                    nc.vector.tensor_add(cnew, cnew, t1[:, :, :n])
                nc.scalar.activation(t1[:, :, :n], cnew, AF.Tanh)
                if ps is not None:
                    do_mm(12, 20)
                    nc.vector.tensor_add(g[:, 12:20, :n], ps[:, 12:20, :n],
                                         PRE_B[:, 12:20, lo:lo + n])
                    nc.scalar.activation(g[:, 12:20, :n], g[:, 12:20, :n],
                                         AF.Sigmoid, scale=GINV)
                px = PRE_B[:, 20:24, lo:lo + n]
                nc.vector.tensor_mul(t2[:, :, :n], g[:, 12:16, :n],
                                     t1[:, :, :n])
                nc.vector.tensor_sub(t2[:, :, :n], t2[:, :, :n], px)
                nc.vector.tensor_mul(t2[:, :, :n], g[:, 16:20, :n],
                                     t2[:, :, :n])
                nc.vector.tensor_add(HB[:, :, lo:lo + n], t2[:, :, :n], px)

            def fwd_gemm_step(lo, n, clo):
                ch = sc_p.tile([128, 8, 65], F8, tag="ch", name="ch")
                lc = sc_p.tile([128, 4, 65], F32, tag="lc", name="lc")
                rc = sc_p.tile([128, 4, 65], F32, tag="rc", name="rc")
                nc.vector.tensor_scalar_mul(ch[:, 0:4, :n],
                                            HCF[:, 0:4, clo:clo + 2 * n - 1:2],
                                            HSCALE)
                nc.vector.tensor_copy(lc[:, :, :n],
                                      HCF[:, 4:8, clo:clo + 2 * n - 1:2])
                nc.vector.tensor_scalar_mul(ch[:, 4:8, :n],
                                            HCF[:, 0:4, clo + 1:clo + 2 * n:2],
                                            HSCALE)
                nc.gpsimd.tensor_copy(rc[:, :, :n],
                                      HCF[:, 4:8, clo + 1:clo + 2 * n:2])
                ps = pr_p.tile([128, 24, 64], F32, tag="rps", name="ps")
                wf_sb = cur["wf"]

                def do_mm(m0, m1):
                    for m in range(m0, m1):
                        for k in range(8):
                            nc.tensor.matmul(
                                ps[:, m, :n],
                                wf_sb[:, (k * 24 + m) * 128:(k * 24 + m + 1) * 128],
                                ch[:, k, :n],
                                start=(k == 0), stop=(k == 7))

                fwd_elem(lo, n, ps, lc[:, :, :n], rc[:, :, :n], do_mm)

            def bwd_gemm_step(lo, n, plo, after=None):
                ch = sc_p.tile([128, 8, 65], F8, tag="ch", name="chb")
                pc = sc_p.tile([128, 4, 65], F32, tag="lc", name="pcb")
                if after is not None:
                    # dependency injection: a throwaway write into ch that
                    # reads `after` holds this step (and the chain behind
                    # it) until `after` is produced — both in the
                    # scheduler's model and on hardware. Keeps the bwd tail
                    # inside the AllGather's latency window instead of
                    # being front-packed before the fwd chain ends.
                    nc.vector.tensor_scalar_mul(ch[:, 0:1, 0:1], after, HSCALE)
                if n == 1:
                    nc.vector.tensor_scalar_mul(ch[:, 0:4, 0:1],
                                                HB[:, :, plo:plo + 1], HSCALE)
                    nc.vector.tensor_copy(pc[:, :, 0:1], CB[:, :, plo:plo + 1])
                else:
                    m2 = n // 2
                    src_h = HB[:, :, plo:plo + m2].unsqueeze(3).broadcast_to(
                        [128, 4, m2, 2])
                    src_c = CB[:, :, plo:plo + m2].unsqueeze(3).broadcast_to(
                        [128, 4, m2, 2])
                    nc.vector.tensor_scalar_mul(
                        ch[:, 0:4, 0:n].rearrange("p c (a b) -> p c a b", b=2),
                        src_h, HSCALE)
                    nc.vector.tensor_copy(
                        pc[:, :, 0:n].rearrange("p c (a b) -> p c a b", b=2), src_c)
                ps = pr_p.tile([128, 24, 64], F32, tag="rps", name="psb")
                wb_sb = cur["wb"]

                def do_mm(m0, m1):
                    for m in range(m0, m1):
                        for k in range(4):
                            nc.tensor.matmul(
                                ps[:, m, :n],
                                wb_sb[:, (k * 20 + m) * 128:(k * 20 + m + 1) * 128],
                                ch[:, k, :n],
                                start=(k == 0), stop=(k == 3))

                bwd_elem(lo, n, ps, pc[:, :, :n], do_mm)

            def consume_gather(ccout, when_ms):
                """Gather-out: one contiguous DMA (Act ring, decoupled from
                the SP weight stream), then 8 PE transposes [8,128]->[128,8]
                into a psum scratch, then copies into the root columns.
                Replaces the old per-element transposing DMAs (a ~15us
                descriptor storm that also dragged the bwd tail with it via
                derived semaphores)."""
                with tc.tile_wait_until(when_ms, enable=False):
                    ccsb = cc_p.tile([8, 1024], F32, tag="ccsb", name="ccsb")
                    nc.scalar.dma_start(ccsb[:, :], ccout[:, :])
                    ptg = pr_p.tile([128, 24, 64], F32, tag="rps", name="ptg")
                    for chn in range(8):
                        nc.tensor.transpose(ptg[:, chn, 0:8],
                                            ccsb[:, chn * 128:(chn + 1) * 128],
                                            ident_sb[0:8, 0:8])
                        if chn % 2 == 0:
                            nc.vector.tensor_copy(HCF[:, chn, 135:143],
                                                  ptg[:, chn, 0:8])
                        else:
                            nc.scalar.activation(HCF[:, chn, 135:143],
                                                 ptg[:, chn, 0:8], AF.Identity)

            cur = enqueue_weights(0, first=True)
            startup_extras()

            for l in range(L):
                bf_sb, bb_sb = cur["bf"], cur["bb"]
                ftile = cur["ft"]

                PRE_F = pre_p.tile([128, 28, NCOL], F32, tag="pref", name="PRE_F")
                PRE_B = pre_p.tile([128, 24, NCOL], F32, tag="preb", name="PRE_B")

                if l > 0:
                    for k in range(8):
                        src = HCF[:, k, :] if k < 4 else HB[:, k - 4, :]
                        nc.vector.tensor_copy(ftile[:, k, :], src)

                # ---- pre-projections: PRE = W_pre @ feats (feature-major) ----
                # the PSUM->PRE moves alternate between the Act and DVE
                # engines so the 2-buffer psum rotation is reader-limited
                # by neither engine alone.
                def emit_pre(groups):
                    for gidx in groups:
                        wpb = cur["wp"][gidx]
                        for mi in range(4):
                            m = gidx * 4 + mi
                            ps = pp_p.tile([128, 143], F32, tag="pps", name="pps")
                            for k in range(8):
                                nc.tensor.matmul(
                                    ps[:],
                                    wpb[:, (k * 4 + mi) * 128:(k * 4 + mi + 1) * 128],
                                    ftile[:, k, :],
                                    start=(k == 0), stop=(k == 7))
                            dst = (PRE_F[:, m, :] if m < 28
                                   else PRE_B[:, m - 28, :])
                            bias = (bf_sb[:, m:m + 1] if m < 28
                                    else bb_sb[:, m - 28:m - 27])
                            if mi % 2 == 0:
                                nc.scalar.activation(dst, ps[:], AF.Identity,
                                                     bias=bias)
                            else:
                                nc.vector.tensor_scalar_add(dst, ps[:], bias)

                emit_pre(GORDER)

                # next layer's weight stream enqueues BEFORE the recurrence's
                # collective-dependent DMAs hit the SP ring
                nxt = enqueue_weights(l + 1, first=False) if l + 1 < L else None

                # ---- recurrences ----
                # fwd chain is the critical path to the AllGather; bwd steps
                # are interleaved so the PE can fill each chain's elementwise
                # latency with the other chain's matmuls.
                # leaves: column 127 (node 1023) first so the node-511 fix
                # gemm below can overlap the remaining 64 leaf columns
                fwd_elem(127, 1, None, None, None)
                fwd_elem(63, 64, None, None, None)  # leaves (slots 63..126)
                bwd_elem(128, 1, None, None)        # root node 0
                # node-511 fix: slot 63 <- left child col 127 (masked), using
                # only the W_l half of wf (k-chunks 0..3). For cores != 0 the
                # mask zeroes the child, making this an idempotent leaf
                # recompute. Must run before the level-8 step below, which
                # consumes slot 63.
                chx = sc_p.tile([128, 8, 65], F8, tag="ch", name="chx")
                lcx = sc_p.tile([128, 4, 65], F32, tag="lc", name="lcx")
                rcx = sc_p.tile([128, 4, 65], F32, tag="rc", name="rcx")
                nc.vector.tensor_scalar(chx[:, 0:4, 0:1], HCF[:, 0:4, 127:128],
                                        HSCALE, mask_sb[:, 0:1],
                                        mybir.AluOpType.mult,
                                        mybir.AluOpType.mult)
                nc.vector.tensor_copy(lcx[:, :, 0:1], HCF[:, 4:8, 127:128])
                nc.vector.tensor_scalar_mul(lcx[:, :, 0:1], lcx[:, :, 0:1],
                                            mask_sb[:, 0:1])
                nc.vector.memset(rcx[:, :, 0:1], 0.0)
                psx = pr_p.tile([128, 24, 64], F32, tag="rps", name="psx")

                def do_mm_fix(m0, m1):
                    for m in range(m0, m1):
                        for k in range(4):
                            nc.tensor.matmul(
                                psx[:, m, 0:1],
                                cur["wf"][:, (k * 24 + m) * 128:(k * 24 + m + 1) * 128],
                                chx[:, k, 0:1], start=(k == 0), stop=(k == 3))

                fwd_elem(63, 1, psx, lcx[:, :, 0:1], rcx[:, :, 0:1], do_mm_fix)
                fwd_gemm_step(31, 32, 63)
                bwd_gemm_step(129, 2, 128)
                fwd_gemm_step(15, 16, 31)
                bwd_gemm_step(131, 4, 129)
                fwd_gemm_step(7, 8, 15)
                bwd_gemm_step(135, 8, 131)
                # the bwd tail starts as soon as the psel root-copy is done,
                # interleaved with the fwd small levels so each chain's
                # matmul bursts fill the other's elementwise tails.
                # copy own root (col 135+c) into local slot 0
                tmp = sc_p.tile([128, 4, 8], F32, tag="pseltmp", name="pseltmp")
                pb = psel_sb[:, :].unsqueeze(1).broadcast_to([128, 4, 8])
                nc.vector.tensor_mul(tmp[:], HB[:, :, 135:143], pb)
                nc.vector.reduce_sum(HB[:, :, 0], tmp[:], mybir.AxisListType.X)
                tmp2 = sc_p.tile([128, 4, 8], F32, tag="pseltmp", name="pseltmp2")
                nc.vector.tensor_mul(tmp2[:], CB[:, :, 135:143], pb)
                nc.vector.reduce_sum(CB[:, :, 0], tmp2[:], mybir.AxisListType.X)
                fwd_gemm_step(3, 4, 7)
                bwd_gemm_step(1, 2, 0)
                fwd_gemm_step(1, 2, 3)
                bwd_gemm_step(3, 4, 1)
                fwd_gemm_step(0, 1, 1)

                # AllGather the 8 subtree roots' (h, c): PE-transpose the
                # [128, 8] root slice to [8, 128], stage in SBUF, then one
                # contiguous DMA (Act ring) to the collective input.
                ccin = dram_p.tile([1024], F32, tag="ccin", name="ccin")
                ccout = dram_p.tile([8, 1024], F32, tag="ccout", name="ccout",
                                    addr_space="Shared")
                with tc.tile_wait_until(0.04, enable=False):
                    pti = pr_p.tile([128, 24, 64], F32, tag="rps", name="pti")
                    ptv = pti[:].rearrange("p a b -> p (a b)")
                    nc.tensor.transpose(ptv[0:8, 0:128], HCF[:, :, 0],
                                        ident_sb[:, :])
                    stg = cc_p.tile([8, 128], F32, tag="ccstg", name="ccstg")
                    nc.vector.tensor_copy(stg[:, :], ptv[0:8, 0:128])
                    nc.scalar.dma_start(
                        ccin[0:1024].rearrange("(g p) -> g p", g=8, p=128),
                        stg[:, :])
                nc.gpsimd.collective_compute(
                    "AllGather", mybir.AluOpType.bypass,
                    ins=[ccin.opt()], outs=[ccout.opt()],
                    replica_groups=[list(range(NCORES))])

                # bwd mid-levels fill the AllGather latency window; then the
                # gather consume + fwd top levels interleave with the rest.
                bwd_gemm_step(7, 8, 3)
                bwd_gemm_step(15, 16, 7)
                bwd_gemm_step(31, 32, 15)
                consume_gather(ccout, 0.041)
                fwd_gemm_step(131, 4, 135)   # top level 2
                fwd_gemm_step(129, 2, 131)   # top level 1
                bwd_gemm_step(63, 64, 31)
                bwd_gemm_step(127, 1, 63)    # node 1023
                fwd_gemm_step(128, 1, 129)   # root

                if nxt is not None:
                    cur = nxt

            # ---- outputs ----
            # split so each piece drains as soon as its columns finalize:
            # HB cols 0:63 are final after bwd(31,32); 63:128 after the
            # node-1023 step; HCF cols after the fwd chain.
            olv = out_loc_d[:].rearrange("(c p) n -> p c n", c=8, p=128)
            nc.sync.dma_start(olv[:, 0:4, :], HCF[:, 0:4, 0:128])
            nc.sync.dma_start(olv[:, 4:8, 0:63], HB[:, :, 0:63])
            nc.sync.dma_start(olv[:, 4:8, 63:127], HB[:, :, 63:127])
            nc.sync.dma_start(olv[:, 4:8, 127:128], HB[:, :, 127:128])

            otv = out_top_d[:].rearrange("(c p) n -> p c n", c=8, p=128)
            nc.sync.dma_start(otv[:, 0:4, :], HCF[:, 0:4, 128:135])
            nc.sync.dma_start(otv[:, 4:8, :], HB[:, :, 128:135])

    nc.finalize()
    return nc


_program_cache = None


def kernel(features, f_px_w, f_px_b, f_x_w, f_x_b, f_l_w, f_l_b, f_r_w, f_r_b,
           b_px_w, b_px_b, b_x_w, b_x_b, b_h_w, b_h_b, left, right, parent):
    global _program_cache, _last_results
    features = np.asarray(features, dtype=np.float32)
    as32 = lambda a: np.asarray(a, dtype=np.float32)

    # ---- host-side packing (DRAM layout == SBUF layout, contiguous DMA) ----
    shared = {}
    for l in range(L):
        # gate rows of the pre-projections carry the x GSCALE fold (the px
        # highway rows stay raw)
        wpre = np.concatenate([_perm_f(as32(f_x_w[l])) * GSCALE, as32(f_px_w[l]),
                               _perm_b(as32(b_x_w[l])) * GSCALE, as32(b_px_w[l])],
                              axis=0)                    # [6656, 1024]
        t = _pack_lhsT(wpre, 8, 52, BF16)                # [8k, 52m, 128p, 128c]
        t = t.reshape(8, 13, 4, 128, 128).transpose(1, 3, 0, 2, 4)
        shared[f"wpre{l}"] = np.ascontiguousarray(t.reshape(13, 128, 4096))
        wrf = _perm_f(np.concatenate([as32(f_l_w[l]), as32(f_r_w[l])], axis=1))
        t = _pack_lhsT(wrf * WSCALE, 8, 24, FP8)         # [8, 24, 128, 128]
        shared[f"wrecf{l}"] = np.ascontiguousarray(
            t.transpose(2, 0, 1, 3).reshape(128, 8 * 24 * 128))
        t = _pack_lhsT(_perm_b(as32(b_h_w[l])) * WSCALE, 4, 20, FP8)
        shared[f"wrecb{l}"] = np.ascontiguousarray(
            t.transpose(2, 0, 1, 3).reshape(128, 4 * 20 * 128))
        bf = np.concatenate([_perm_f(as32(f_x_b[l]) + as32(f_l_b[l])
                                     + as32(f_r_b[l])) * GSCALE,
                             as32(f_px_b[l])])           # [3584]
        shared[f"biasf{l}"] = np.ascontiguousarray(bf.reshape(28, 128).T)
        bb = np.concatenate([_perm_b(as32(b_x_b[l]) + as32(b_h_b[l])) * GSCALE,
                             as32(b_px_b[l])])
        shared[f"biasb{l}"] = np.ascontiguousarray(bb.reshape(24, 128).T)
    shared["ident"] = np.eye(128, dtype=np.float32)

    in_maps = []
    ids_all = []
    for c in range(NCORES):
        ids = _node_ids(c)
        ids_all.append(ids)
        ft = features[ids].T.astype(BF16)                # [1024, 143]
        m = {k: v for k, v in shared.items()}
        m["featsT"] = np.ascontiguousarray(
            ft.reshape(8, 128, NCOL).transpose(1, 0, 2).reshape(128, 8 * NCOL))
        m["mask"] = np.full((128, 1), 1.0 if c == 0 else 0.0, np.float32)
        ps = np.zeros((128, 8), np.float32)
        ps[:, c] = 1.0
        m["psel"] = ps
        in_maps.append(m)

    if _program_cache is None:
        _program_cache = _build_program()
    nc = _program_cache

    trace = bool(os.environ.get("KERNEL_TRACE"))
    tdir = os.environ.get("KERNEL_TRACE_DIR") or None
    res = run_bass_kernel_spmd(nc, in_maps, core_ids=list(range(NCORES)),
                               trace=trace, tmpdir=tdir)
    _last_results = res

    out = np.empty((N, 2 * H), np.float32)
    for c in range(NCORES):
        loc = res.results[c]["out_loc"]                  # [1024, 128]
        nloc = 128 if c == 0 else 127
        out[ids_all[c][0:nloc]] = loc[:, 0:nloc].T
    out[0:7] = res.results[0]["out_top"].T
    return out



# revision 85
# speedup vs baseline: 1.2203x; 1.2203x over previous
"""Multi-layer bidirectional Tree-LSTM on 8 TRN2 NeuronCores.

Strategy: the input is a complete binary tree of 1024 nodes. Below level 3
there are 8 independent subtrees (rooted at nodes 7..14) -> one subtree per
core (data parallel). The top 7 nodes (0..6) are computed replicated on all
cores; one small AllGather per layer exchanges the 8 subtree-root (h, c)
pairs for the leaves->root direction.

On-device layout is feature-major (hidden dim on partitions, nodes on the
free axis); weights are stationary and node columns stream, so no
transposes are needed anywhere in the recurrence.

Per-core column layout (143 columns):
  0..126   : BFS slots of subtree(7+c)  (slot s, level k=floor(log2(s+1)))
  127      : node 1023 (replicated on every core; only core 0's is used)
  128..134 : top nodes 0..6 (replicated)
  135..142 : subtree roots 7..14 (fwd: from AllGather; bwd: replicated)

Precision: pre-projection weights bf16; recurrence weights fp8 e4m3
scaled by 64 (the recurrence is LDWEIGHTS-bandwidth-bound and fp8 FWL
loads 2x faster than bf16; the 1/64 un-scale is folded into the bf16
child/parent-h copies, which is a lossless exponent shift). Measured
end-to-end rel err ~6e-3 vs the 2e-2 gate.

Gate row order is permuted at pack time to pipeline the gate activations
into the level matmul phase: fwd [u ig fl fr og r] (tanh 0:4, sigmoid
4:24), bwd [u ig f og r] (tanh 0:4, sigmoid 4:20). u comes first so
tanh(u) fires after the first 4 m-chunks accumulate; og/r last (their
consumers sit at the end of the elementwise chain).

All weight DRAM tensors are host-packed to the exact SBUF layout so every
weight DMA is a plain contiguous [128, X] copy (large descriptors, full
HBM bandwidth). The layer pipeline enqueues layer l+1's full weight
stream on the SP DMA ring during layer l's pre-projections; all
collective-path DMAs (ccin write, gather read) and small constants ride
the Activation-engine DMA ring so they never queue behind the weight
stream. The AllGather input/output transposes run on the PE against a
staged identity (one contiguous DMA each way) instead of the old
per-element transposing DMAs. The bwd tail is emitted interleaved with
the fwd small levels and the post-AllGather fwd top levels, so each
chain's matmul bursts fill the other chain's elementwise latency and
the AllGather window stays full.
"""

import os
import sys

for _p in ("/opt/trn_rl_repo",):
    if _p not in sys.path and os.path.isdir(_p):
        sys.path.insert(0, _p)

import numpy as np
import ml_dtypes

try:
    import jax
    jax.config.update("jax_compilation_cache_dir", os.environ.get("KERNEL_JAX_CACHE", "/tmp/jax_neff_cache"))
    jax.config.update("jax_persistent_cache_min_compile_time_secs", 5.0)
    jax.config.update("jax_persistent_cache_min_entry_size_bytes", 0)
except Exception:
    pass

import concourse.bass as bass
import concourse.mybir as mybir
from concourse import bacc
from concourse.tile import TileContext
from concourse.bass_utils import run_bass_kernel_spmd

BF16 = ml_dtypes.bfloat16
FP8 = ml_dtypes.float8_e4m3fn
F32 = mybir.dt.float32
B16 = mybir.dt.bfloat16
F8 = mybir.dt.float8e4
AF = mybir.ActivationFunctionType

N, D, H, L = 1024, 1024, 512, 2
NCOL = 143  # 127 subtree + node1023 + 7 top + 8 roots
NCORES = 8
# fp8 recurrence scales: weights x64, h operand x4. The product (x256) is
# folded into the pre-projection gate rows/biases on the host, and undone
# by the activation `scale` parameter (powers of two -> exact).
WSCALE = 64.0
HSCALE = 4.0
GSCALE = WSCALE * HSCALE
GINV = 1.0 / GSCALE
# pre-projection group streaming order: PRE_F groups (0-6) first so the
# fwd chain (the long chain: leaves -> root -> AllGather -> top levels)
# can start as soon as its half of PRE is done.
GORDER = [0, 1, 2, 3, 4, 5, 6, 7, 8, 9, 10, 11, 12]

_last_results = None  # stashed BassKernelResults for test.py


def _node_ids(c):
    ids = []
    for k in range(7):
        base = (8 + c) * (1 << k) - 1
        ids.extend(range(base, base + (1 << k)))
    ids.append(1023)
    ids.extend(range(0, 7))
    ids.extend(range(7, 15))
    return np.asarray(ids, dtype=np.int64)


def _pack_lhsT(w, kchunks, mchunks, dty):
    # w: [M, K] fp32 -> lhsT tiles [kchunks, mchunks, 128, 128] where
    # tile[k, m, kp, mc] = w[m*128+mc, k*128+kp]
    Mdim, Kdim = w.shape
    assert Mdim == mchunks * 128 and Kdim == kchunks * 128
    t = w.reshape(mchunks, 128, kchunks, 128).transpose(2, 0, 3, 1)
    return np.ascontiguousarray(t.astype(dty))


def _perm_f(w):
    # fwd gate rows [ig og fl fr u r] -> [u ig fl fr og r]: u first so
    # tanh(u) fires after 4 m-chunks of the level matmul; og/r last (their
    # consumers are at the end of the elementwise chain anyway)
    return np.concatenate([w[2048:2560], w[0:512], w[1024:1536],
                           w[1536:2048], w[512:1024], w[2560:3072]], axis=0)


def _perm_b(w):
    # bwd gate rows [ig og f u r] -> [u ig f og r]
    return np.concatenate([w[1536:2048], w[0:512], w[1024:1536],
                           w[512:1024], w[2048:2560]], axis=0)


def _build_program():
    nc = bacc.Bacc("TRN2", target_bir_lowering=False, debug=False,
                   num_devices=NCORES)

    featsT_d = nc.dram_tensor("featsT", [128, 8 * NCOL], B16, kind="ExternalInput")
    wpre_d, wrecf_d, wrecb_d, biasf_d, biasb_d = [], [], [], [], []
    for l in range(L):
        wpre_d.append(nc.dram_tensor(f"wpre{l}", [13, 128, 4096], B16,
                                     kind="ExternalInput"))
        wrecf_d.append(nc.dram_tensor(f"wrecf{l}", [128, 8 * 24 * 128], F8,
                                      kind="ExternalInput"))
        wrecb_d.append(nc.dram_tensor(f"wrecb{l}", [128, 4 * 20 * 128], F8,
                                      kind="ExternalInput"))
        biasf_d.append(nc.dram_tensor(f"biasf{l}", [128, 28], F32,
                                      kind="ExternalInput"))
        biasb_d.append(nc.dram_tensor(f"biasb{l}", [128, 24], F32,
                                      kind="ExternalInput"))
    mask_d = nc.dram_tensor("mask", [128, 1], F32, kind="ExternalInput")
    psel_d = nc.dram_tensor("psel", [128, 8], F32, kind="ExternalInput")
    ident_d = nc.dram_tensor("ident", [128, 128], F32, kind="ExternalInput")
    out_loc_d = nc.dram_tensor("out_loc", [1024, 128], F32, kind="ExternalOutput")
    out_top_d = nc.dram_tensor("out_top", [1024, 7], F32, kind="ExternalOutput")

    with TileContext(nc) as tc:
        with (
            tc.tile_pool(name="state", bufs=1) as state_p,
            tc.tile_pool(name="weights", bufs=2) as w_p,
            # wb is single-buffered: layer l+1's wb DMA waits for layer l's
            # last bwd-step reader (~recurrence end) and lands well before
            # layer l+1's first bwd gemm. The freed SBUF pays for a deeper
            # wpre-group rotation (fewer trickling groups in layer 1's pre).
            tc.tile_pool(name="weights_b", bufs=1) as wb_p,
            tc.tile_pool(name="pre", bufs=1) as pre_p,
            tc.tile_pool(name="wstream", bufs=10) as ws_p,
            tc.tile_pool(name="scratch", bufs=2) as sc_p,
            tc.tile_pool(name="ccstage", bufs=1) as cc_p,
            tc.tile_pool(name="psum_pre", bufs=2, space="PSUM") as pp_p,
            tc.tile_pool(name="psum_rec", bufs=2, space="PSUM") as pr_p,
            tc.tile_pool(name="dram", bufs=1, space="DRAM") as dram_p,
        ):
            # HCF holds fwd h (chunks 0:4) and c (chunks 4:8) in one tile so
            # the collective input (h,c of the subtree root) is a single
            # [128, 8] slice -> one PE transpose -> one contiguous DMA.
            HCF = state_p.tile([128, 8, NCOL], F32, name="HCF")
            HB = state_p.tile([128, 4, NCOL], F32, name="HB")
            CB = state_p.tile([128, 4, NCOL], F32, name="CB")
            mask_sb = state_p.tile([128, 1], F32, name="mask_sb")
            psel_sb = state_p.tile([128, 8], F32, name="psel_sb")
            ident_sb = state_p.tile([128, 128], F32, name="ident_sb")

            def startup_extras():
                """Emitted after layer 0's critical weight DMAs so these
                don't occupy SP-ring slots ahead of the first matmul's
                inputs (small constants ride the Act ring). Includes the
                warmup collective: the first AllGather of a run costs ~3-5x
                the steady-state latency, so absorb it during initial weight
                streaming (CC cores are idle there)."""
                nc.scalar.dma_start(mask_sb[:], mask_d[:])
                nc.scalar.dma_start(psel_sb[:], psel_d[:])
                nc.scalar.dma_start(ident_sb[:], ident_d[:])
                ccw_in = dram_p.tile([8], F32, tag="ccwi", name="ccw_in")
                ccw_out = dram_p.tile([8, 8], F32, tag="ccwo", name="ccw_out",
                                      addr_space="Shared")
                nc.scalar.dma_start(ccw_in[0:8], psel_d[0, 0:8])
                nc.gpsimd.collective_compute(
                    "AllGather", mybir.AluOpType.bypass,
                    ins=[ccw_in.opt()], outs=[ccw_out.opt()],
                    replica_groups=[list(range(NCORES))])

            # current-layer tile handles (set by enqueue_weights)
            cur = {}

            def enqueue_weights(l, first):
                """Allocate layer-l weight tiles and enqueue all their DMAs
                on the SP ring. Order within the layer: biases, (feats),
                wpre groups 0-4, recurrence weights, wpre groups 5-12."""
                st = {}
                st["bf"] = w_p.tile([128, 28], F32, tag="bf", name="bf_sb")
                st["bb"] = w_p.tile([128, 24], F32, tag="bb", name="bb_sb")
                # Act ring: keeps these small descriptor-heavy copies out of
                # the SP ring ahead of the first weight group
                nc.scalar.dma_start(st["bf"][:], biasf_d[l][:])
                nc.scalar.dma_start(st["bb"][:], biasb_d[l][:])
                st["ft"] = pre_p.tile([128, 8, NCOL], B16, tag="ft", name="ftile")
                if first:
                    # Act ring: idle at startup, overlaps the SP weight stream
                    nc.scalar.dma_start(
                        st["ft"][:].rearrange("p k c -> p (k c)"), featsT_d[:])
                st["wp"] = {}
                st["wf"] = w_p.tile([128, 8 * 24 * 128], F8, tag="wf", name="wf_sb")
                st["wb"] = wb_p.tile([128, 4 * 20 * 128], F8, tag="wb", name="wb_sb")
                # PRE_B groups (7..12) stream first: their consumer (the bwd
                # root step) is early in the interleaved recurrence, so the
                # scheduler keeps their matmuls early and the next layer's
                # group DMAs (buffer-WAR on these readers) can prefetch.
                for i, gidx in enumerate(GORDER):
                    wpb = ws_p.tile([128, 8 * 4 * 128], B16, tag="wpre", name="wpb")
                    if first and i == 0:
                        # split the first-consumed group into k-part pieces
                        # so the very first matmuls (which read only k-chunk
                        # 0) start before the whole 1MB group lands
                        for kq in range(4):
                            nc.sync.dma_start(
                                wpb[:, kq * 1024:(kq + 1) * 1024],
                                wpre_d[l][gidx][:, kq * 1024:(kq + 1) * 1024])
                    else:
                        nc.sync.dma_start(wpb[:], wpre_d[l][gidx])
                    st["wp"][gidx] = wpb
                    if i == 8:
                        nc.sync.dma_start(st["wf"][:], wrecf_d[l][:])
                        nc.sync.dma_start(st["wb"][:], wrecb_d[l][:])
                return st

            def fwd_elem(lo, n, ps, lc, rc, do_mm=None):
                """gates -> (c, hf) for fwd columns [lo, lo+n).
                gate order: u ig fl fr og r  (tanh 0:4, sigmoid 4:24).
                Gate pre-activations are carried x GSCALE; the activation
                scale undoes it exactly. When do_mm is given, the level's
                matmuls are EMITTED in three m-chunk pieces interleaved
                with the gate activations, so each activation piece fires
                while later m-chunks are still accumulating and most of
                the elementwise chain hides under the matmul phase."""
                g = sc_p.tile([128, 24, 65], F32, tag="gates", name="g")
                cnew = HCF[:, 4:8, lo:lo + n]
                t1 = sc_p.tile([128, 4, 65], F32, tag="t1", name="t1")
                t2 = sc_p.tile([128, 4, 65], F32, tag="t2", name="t2")
                if ps is None:
                    nc.scalar.activation(g[:, 0:4, :n], PRE_F[:, 0:4, lo:lo + n],
                                         AF.Tanh, scale=GINV)
                    nc.scalar.activation(g[:, 4:24, :n], PRE_F[:, 4:24, lo:lo + n],
                                         AF.Sigmoid, scale=GINV)
                else:
                    do_mm(0, 4)
                    nc.vector.tensor_add(g[:, 0:4, :n], ps[:, 0:4, :n],
                                         PRE_F[:, 0:4, lo:lo + n])
                    nc.scalar.activation(g[:, 0:4, :n], g[:, 0:4, :n], AF.Tanh,
                                         scale=GINV)
                    do_mm(4, 16)
                    nc.vector.tensor_add(g[:, 4:16, :n], ps[:, 4:16, :n],
                                         PRE_F[:, 4:16, lo:lo + n])
                    nc.scalar.activation(g[:, 4:16, :n], g[:, 4:16, :n],
                                         AF.Sigmoid, scale=GINV)
                # c = ig*u (+ fl*lc + fr*rc)
                nc.vector.tensor_mul(cnew, g[:, 4:8, :n], g[:, 0:4, :n])
                if lc is not None:
                    # fr*rc runs on the (otherwise idle) Pool engine in
                    # parallel with fl*lc on DVE
                    nc.gpsimd.tensor_mul(t2[:, :, :n], g[:, 12:16, :n], rc)
                    nc.vector.tensor_mul(t1[:, :, :n], g[:, 8:12, :n], lc)
                    nc.vector.tensor_add(cnew, cnew, t1[:, :, :n])
                    nc.vector.tensor_add(cnew, cnew, t2[:, :, :n])
                nc.scalar.activation(t1[:, :, :n], cnew, AF.Tanh)
                if ps is not None:
                    do_mm(16, 24)
                    nc.vector.tensor_add(g[:, 16:24, :n], ps[:, 16:24, :n],
                                         PRE_F[:, 16:24, lo:lo + n])
                    nc.scalar.activation(g[:, 16:24, :n], g[:, 16:24, :n],
                                         AF.Sigmoid, scale=GINV)
                # hf = og*tanh(c)*r + (1-r)*px = r*(hh - px) + px
                px = PRE_F[:, 24:28, lo:lo + n]
                nc.vector.tensor_mul(t2[:, :, :n], g[:, 16:20, :n],
                                     t1[:, :, :n])
                nc.vector.tensor_sub(t2[:, :, :n], t2[:, :, :n], px)
                nc.vector.tensor_mul(t2[:, :, :n], g[:, 20:24, :n],
                                     t2[:, :, :n])
                nc.vector.tensor_add(HCF[:, 0:4, lo:lo + n], t2[:, :, :n], px)

            def bwd_elem(lo, n, ps, pc, do_mm=None):
                # gate order: u ig f og r  (tanh 0:4, sigmoid 4:20), split
                # into pieces matching the matmul m-chunk order
                g = sc_p.tile([128, 24, 65], F32, tag="gates", name="gb")
                cnew = CB[:, :, lo:lo + n]
                t1 = sc_p.tile([128, 4, 65], F32, tag="t1", name="t1b")
                t2 = sc_p.tile([128, 4, 65], F32, tag="t2", name="t2b")
                if ps is None:
                    nc.scalar.activation(g[:, 0:4, :n], PRE_B[:, 0:4, lo:lo + n],
                                         AF.Tanh, scale=GINV)
                    nc.scalar.activation(g[:, 4:20, :n], PRE_B[:, 4:20, lo:lo + n],
                                         AF.Sigmoid, scale=GINV)
                else:
                    do_mm(0, 4)
                    nc.vector.tensor_add(g[:, 0:4, :n], ps[:, 0:4, :n],
                                         PRE_B[:, 0:4, lo:lo + n])
                    nc.scalar.activation(g[:, 0:4, :n], g[:, 0:4, :n], AF.Tanh,
                                         scale=GINV)
                    do_mm(4, 12)
                    nc.vector.tensor_add(g[:, 4:12, :n], ps[:, 4:12, :n],
                                         PRE_B[:, 4:12, lo:lo + n])
                    nc.scalar.activation(g[:, 4:12, :n], g[:, 4:12, :n],
                                         AF.Sigmoid, scale=GINV)
                if pc is not None:
                    nc.gpsimd.tensor_mul(t1[:, :, :n], g[:, 8:12, :n], pc)
                nc.vector.tensor_mul(cnew, g[:, 4:8, :n], g[:, 0:4, :n])  # ig*u
                if pc is not None:
                    nc.vector.tensor_add(cnew, cnew, t1[:, :, :n])
                nc.scalar.activation(t1[:, :, :n], cnew, AF.Tanh)
                if ps is not None:
                    do_mm(12, 20)
                    nc.vector.tensor_add(g[:, 12:20, :n], ps[:, 12:20, :n],
                                         PRE_B[:, 12:20, lo:lo + n])
                    nc.scalar.activation(g[:, 12:20, :n], g[:, 12:20, :n],
                                         AF.Sigmoid, scale=GINV)
                px = PRE_B[:, 20:24, lo:lo + n]
                nc.vector.tensor_mul(t2[:, :, :n], g[:, 12:16, :n],
                                     t1[:, :, :n])
                nc.vector.tensor_sub(t2[:, :, :n], t2[:, :, :n], px)
                nc.vector.tensor_mul(t2[:, :, :n], g[:, 16:20, :n],
                                     t2[:, :, :n])
                nc.vector.tensor_add(HB[:, :, lo:lo + n], t2[:, :, :n], px)

            def fwd_gemm_step(lo, n, clo):
                ch = sc_p.tile([128, 8, 65], F8, tag="ch", name="ch")
                lc = sc_p.tile([128, 4, 65], F32, tag="lc", name="lc")
                rc = sc_p.tile([128, 4, 65], F32, tag="rc", name="rc")
                nc.vector.tensor_scalar_mul(ch[:, 0:4, :n],
                                            HCF[:, 0:4, clo:clo + 2 * n - 1:2],
                                            HSCALE)
                nc.vector.tensor_copy(lc[:, :, :n],
                                      HCF[:, 4:8, clo:clo + 2 * n - 1:2])
                nc.vector.tensor_scalar_mul(ch[:, 4:8, :n],
                                            HCF[:, 0:4, clo + 1:clo + 2 * n:2],
                                            HSCALE)
                nc.gpsimd.tensor_copy(rc[:, :, :n],
                                      HCF[:, 4:8, clo + 1:clo + 2 * n:2])
                ps = pr_p.tile([128, 24, 64], F32, tag="rps", name="ps")
                wf_sb = cur["wf"]

                def do_mm(m0, m1):
                    for m in range(m0, m1):
                        for k in range(8):
                            nc.tensor.matmul(
                                ps[:, m, :n],
                                wf_sb[:, (k * 24 + m) * 128:(k * 24 + m + 1) * 128],
                                ch[:, k, :n],
                                start=(k == 0), stop=(k == 7))

                fwd_elem(lo, n, ps, lc[:, :, :n], rc[:, :, :n], do_mm)

            def bwd_gemm_step(lo, n, plo, after=None):
                ch = sc_p.tile([128, 8, 65], F8, tag="ch", name="chb")
                pc = sc_p.tile([128, 4, 65], F32, tag="lc", name="pcb")
                if after is not None:
                    # dependency injection: a throwaway write into ch that
                    # reads `after` holds this step (and the chain behind
                    # it) until `after` is produced — both in the
                    # scheduler's model and on hardware. Keeps the bwd tail
                    # inside the AllGather's latency window instead of
                    # being front-packed before the fwd chain ends.
                    nc.vector.tensor_scalar_mul(ch[:, 0:1, 0:1], after, HSCALE)
                if n == 1:
                    nc.vector.tensor_scalar_mul(ch[:, 0:4, 0:1],
                                                HB[:, :, plo:plo + 1], HSCALE)
                    nc.vector.tensor_copy(pc[:, :, 0:1], CB[:, :, plo:plo + 1])
                else:
                    m2 = n // 2
                    src_h = HB[:, :, plo:plo + m2].unsqueeze(3).broadcast_to(
                        [128, 4, m2, 2])
                    src_c = CB[:, :, plo:plo + m2].unsqueeze(3).broadcast_to(
                        [128, 4, m2, 2])
                    nc.vector.tensor_scalar_mul(
                        ch[:, 0:4, 0:n].rearrange("p c (a b) -> p c a b", b=2),
                        src_h, HSCALE)
                    nc.vector.tensor_copy(
                        pc[:, :, 0:n].rearrange("p c (a b) -> p c a b", b=2), src_c)
                ps = pr_p.tile([128, 24, 64], F32, tag="rps", name="psb")
                wb_sb = cur["wb"]

                def do_mm(m0, m1):
                    for m in range(m0, m1):
                        for k in range(4):
                            nc.tensor.matmul(
                                ps[:, m, :n],
                                wb_sb[:, (k * 20 + m) * 128:(k * 20 + m + 1) * 128],
                                ch[:, k, :n],
                                start=(k == 0), stop=(k == 3))

                bwd_elem(lo, n, ps, pc[:, :, :n], do_mm)

            def consume_gather(ccout, when_ms):
                """Gather-out: one contiguous DMA (Act ring, decoupled from
                the SP weight stream), then 8 PE transposes [8,128]->[128,8]
                into a psum scratch, then copies into the root columns.
                Replaces the old per-element transposing DMAs (a ~15us
                descriptor storm that also dragged the bwd tail with it via
                derived semaphores)."""
                with tc.tile_wait_until(when_ms, enable=False):
                    ccsb = cc_p.tile([8, 1024], F32, tag="ccsb", name="ccsb")
                    nc.scalar.dma_start(ccsb[:, :], ccout[:, :])
                    ptg = pr_p.tile([128, 24, 64], F32, tag="rps", name="ptg")
                    for chn in range(8):
                        nc.tensor.transpose(ptg[:, chn, 0:8],
                                            ccsb[:, chn * 128:(chn + 1) * 128],
                                            ident_sb[0:8, 0:8])
                        if chn % 2 == 0:
                            nc.vector.tensor_copy(HCF[:, chn, 135:143],
                                                  ptg[:, chn, 0:8])
                        else:
                            nc.scalar.activation(HCF[:, chn, 135:143],
                                                 ptg[:, chn, 0:8], AF.Identity)

            cur = enqueue_weights(0, first=True)
            startup_extras()

            for l in range(L):
                bf_sb, bb_sb = cur["bf"], cur["bb"]
                ftile = cur["ft"]

                PRE_F = pre_p.tile([128, 28, NCOL], F32, tag="pref", name="PRE_F")
                PRE_B = pre_p.tile([128, 24, NCOL], F32, tag="preb", name="PRE_B")

                if l > 0:
                    for k in range(8):
                        src = HCF[:, k, :] if k < 4 else HB[:, k - 4, :]
                        nc.vector.tensor_copy(ftile[:, k, :], src)

                # ---- pre-projections: PRE = W_pre @ feats (feature-major) ----
                # the PSUM->PRE moves alternate between the Act and DVE
                # engines so the 2-buffer psum rotation is reader-limited
                # by neither engine alone.
                def emit_pre(groups):
                    for gidx in groups:
                        wpb = cur["wp"][gidx]
                        for mi in range(4):
                            m = gidx * 4 + mi
                            ps = pp_p.tile([128, 143], F32, tag="pps", name="pps")
                            for k in range(8):
                                nc.tensor.matmul(
                                    ps[:],
                                    wpb[:, (k * 4 + mi) * 128:(k * 4 + mi + 1) * 128],
                                    ftile[:, k, :],
                                    start=(k == 0), stop=(k == 7))
                            dst = (PRE_F[:, m, :] if m < 28
                                   else PRE_B[:, m - 28, :])
                            bias = (bf_sb[:, m:m + 1] if m < 28
                                    else bb_sb[:, m - 28:m - 27])
                            if mi % 2 == 0:
                                nc.scalar.activation(dst, ps[:], AF.Identity,
                                                     bias=bias)
                            else:
                                nc.vector.tensor_scalar_add(dst, ps[:], bias)

                emit_pre(GORDER)

                # next layer's weight stream enqueues BEFORE the recurrence's
                # collective-dependent DMAs hit the SP ring
                nxt = enqueue_weights(l + 1, first=False) if l + 1 < L else None

                # ---- recurrences ----
                # fwd chain is the critical path to the AllGather; bwd steps
                # are interleaved so the PE can fill each chain's elementwise
                # latency with the other chain's matmuls.
                # leaves: column 127 (node 1023) first so the node-511 fix
                # gemm below can overlap the remaining 64 leaf columns
                fwd_elem(127, 1, None, None, None)
                fwd_elem(63, 64, None, None, None)  # leaves (slots 63..126)
                bwd_elem(128, 1, None, None)        # root node 0
                # node-511 fix: slot 63 <- left child col 127 (masked), using
                # only the W_l half of wf (k-chunks 0..3). For cores != 0 the
                # mask zeroes the child, making this an idempotent leaf
                # recompute. Must run before the level-8 step below, which
                # consumes slot 63.
                chx = sc_p.tile([128, 8, 65], F8, tag="ch", name="chx")
                lcx = sc_p.tile([128, 4, 65], F32, tag="lc", name="lcx")
                rcx = sc_p.tile([128, 4, 65], F32, tag="rc", name="rcx")
                nc.vector.tensor_scalar(chx[:, 0:4, 0:1], HCF[:, 0:4, 127:128],
                                        HSCALE, mask_sb[:, 0:1],
                                        mybir.AluOpType.mult,
                                        mybir.AluOpType.mult)
                nc.vector.tensor_copy(lcx[:, :, 0:1], HCF[:, 4:8, 127:128])
                nc.vector.tensor_scalar_mul(lcx[:, :, 0:1], lcx[:, :, 0:1],
                                            mask_sb[:, 0:1])
                nc.vector.memset(rcx[:, :, 0:1], 0.0)
                psx = pr_p.tile([128, 24, 64], F32, tag="rps", name="psx")

                def do_mm_fix(m0, m1):
                    for m in range(m0, m1):
                        for k in range(4):
                            nc.tensor.matmul(
                                psx[:, m, 0:1],
                                cur["wf"][:, (k * 24 + m) * 128:(k * 24 + m + 1) * 128],
                                chx[:, k, 0:1], start=(k == 0), stop=(k == 3))

                fwd_elem(63, 1, psx, lcx[:, :, 0:1], rcx[:, :, 0:1], do_mm_fix)
                fwd_gemm_step(31, 32, 63)
                bwd_gemm_step(129, 2, 128)
                fwd_gemm_step(15, 16, 31)
                bwd_gemm_step(131, 4, 129)
                fwd_gemm_step(7, 8, 15)
                bwd_gemm_step(135, 8, 131)
                # the bwd tail starts as soon as the psel root-copy is done,
                # interleaved with the fwd small levels so each chain's
                # matmul bursts fill the other's elementwise tails.
                # copy own root (col 135+c) into local slot 0
                tmp = sc_p.tile([128, 4, 8], F32, tag="pseltmp", name="pseltmp")
                pb = psel_sb[:, :].unsqueeze(1).broadcast_to([128, 4, 8])
                nc.vector.tensor_mul(tmp[:], HB[:, :, 135:143], pb)
                nc.vector.reduce_sum(HB[:, :, 0], tmp[:], mybir.AxisListType.X)
                tmp2 = sc_p.tile([128, 4, 8], F32, tag="pseltmp", name="pseltmp2")
                nc.vector.tensor_mul(tmp2[:], CB[:, :, 135:143], pb)
                nc.vector.reduce_sum(CB[:, :, 0], tmp2[:], mybir.AxisListType.X)
                fwd_gemm_step(3, 4, 7)
                bwd_gemm_step(1, 2, 0)
                fwd_gemm_step(1, 2, 3)
                bwd_gemm_step(3, 4, 1)
                fwd_gemm_step(0, 1, 1)

                # AllGather the 8 subtree roots' (h, c): PE-transpose the
                # [128, 8] root slice to [8, 128], stage in SBUF, then one
                # contiguous DMA (Act ring) to the collective input.
                ccin = dram_p.tile([1024], F32, tag="ccin", name="ccin")
                ccout = dram_p.tile([8, 1024], F32, tag="ccout", name="ccout",
                                    addr_space="Shared")
                with tc.tile_wait_until(0.04, enable=False):
                    pti = pr_p.tile([128, 24, 64], F32, tag="rps", name="pti")
                    ptv = pti[:].rearrange("p a b -> p (a b)")
                    nc.tensor.transpose(ptv[0:8, 0:128], HCF[:, :, 0],
                                        ident_sb[:, :])
                    stg = cc_p.tile([8, 128], F32, tag="ccstg", name="ccstg")
                    nc.vector.tensor_copy(stg[:, :], ptv[0:8, 0:128])
                    nc.scalar.dma_start(
                        ccin[0:1024].rearrange("(g p) -> g p", g=8, p=128),
                        stg[:, :])
                nc.gpsimd.collective_compute(
                    "AllGather", mybir.AluOpType.bypass,
                    ins=[ccin.opt()], outs=[ccout.opt()],
                    replica_groups=[list(range(NCORES))])

                # bwd mid-levels fill the AllGather latency window; then the
                # gather consume + fwd top levels interleave with the rest.
                bwd_gemm_step(7, 8, 3)
                bwd_gemm_step(15, 16, 7)
                bwd_gemm_step(31, 32, 15)
                consume_gather(ccout, 0.041)
                fwd_gemm_step(131, 4, 135)   # top level 2
                fwd_gemm_step(129, 2, 131)   # top level 1
                bwd_gemm_step(63, 64, 31)
                bwd_gemm_step(127, 1, 63)    # node 1023
                fwd_gemm_step(128, 1, 129)   # root

                if nxt is not None:
                    cur = nxt

            # ---- outputs ----
            # split so each piece drains as soon as its columns finalize:
            # HB cols 0:63 are final after bwd(31,32); 63:128 after the
            # node-1023 step; HCF cols after the fwd chain.
            olv = out_loc_d[:].rearrange("(c p) n -> p c n", c=8, p=128)
            nc.sync.dma_start(olv[:, 0:4, :], HCF[:, 0:4, 0:128])
            nc.sync.dma_start(olv[:, 4:8, 0:63], HB[:, :, 0:63])
            nc.sync.dma_start(olv[:, 4:8, 63:127], HB[:, :, 63:127])
            nc.sync.dma_start(olv[:, 4:8, 127:128], HB[:, :, 127:128])

            otv = out_top_d[:].rearrange("(c p) n -> p c n", c=8, p=128)
            nc.sync.dma_start(otv[:, 0:4, :], HCF[:, 0:4, 128:135])
            nc.sync.dma_start(otv[:, 4:8, :], HB[:, :, 128:135])

    nc.finalize()
    return nc


_program_cache = None


def kernel(features, f_px_w, f_px_b, f_x_w, f_x_b, f_l_w, f_l_b, f_r_w, f_r_b,
           b_px_w, b_px_b, b_x_w, b_x_b, b_h_w, b_h_b, left, right, parent):
    global _program_cache, _last_results
    features = np.asarray(features, dtype=np.float32)
    as32 = lambda a: np.asarray(a, dtype=np.float32)

    # ---- host-side packing (DRAM layout == SBUF layout, contiguous DMA) ----
    shared = {}
    for l in range(L):
        # gate rows of the pre-projections carry the x GSCALE fold (the px
        # highway rows stay raw)
        wpre = np.concatenate([_perm_f(as32(f_x_w[l])) * GSCALE, as32(f_px_w[l]),
                               _perm_b(as32(b_x_w[l])) * GSCALE, as32(b_px_w[l])],
                              axis=0)                    # [6656, 1024]
        t = _pack_lhsT(wpre, 8, 52, BF16)                # [8k, 52m, 128p, 128c]
        t = t.reshape(8, 13, 4, 128, 128).transpose(1, 3, 0, 2, 4)
        shared[f"wpre{l}"] = np.ascontiguousarray(t.reshape(13, 128, 4096))
        wrf = _perm_f(np.concatenate([as32(f_l_w[l]), as32(f_r_w[l])], axis=1))
        t = _pack_lhsT(wrf * WSCALE, 8, 24, FP8)         # [8, 24, 128, 128]
        shared[f"wrecf{l}"] = np.ascontiguousarray(
            t.transpose(2, 0, 1, 3).reshape(128, 8 * 24 * 128))
        t = _pack_lhsT(_perm_b(as32(b_h_w[l])) * WSCALE, 4, 20, FP8)
        shared[f"wrecb{l}"] = np.ascontiguousarray(
            t.transpose(2, 0, 1, 3).reshape(128, 4 * 20 * 128))
        bf = np.concatenate([_perm_f(as32(f_x_b[l]) + as32(f_l_b[l])
                                     + as32(f_r_b[l])) * GSCALE,
                             as32(f_px_b[l])])           # [3584]
        shared[f"biasf{l}"] = np.ascontiguousarray(bf.reshape(28, 128).T)
        bb = np.concatenate([_perm_b(as32(b_x_b[l]) + as32(b_h_b[l])) * GSCALE,
                             as32(b_px_b[l])])
        shared[f"biasb{l}"] = np.ascontiguousarray(bb.reshape(24, 128).T)
    shared["ident"] = np.eye(128, dtype=np.float32)

    in_maps = []
    ids_all = []
    for c in range(NCORES):
        ids = _node_ids(c)
        ids_all.append(ids)
        ft = features[ids].T.astype(BF16)                # [1024, 143]
        m = {k: v for k, v in shared.items()}
        m["featsT"] = np.ascontiguousarray(
            ft.reshape(8, 128, NCOL).transpose(1, 0, 2).reshape(128, 8 * NCOL))
        m["mask"] = np.full((128, 1), 1.0 if c == 0 else 0.0, np.float32)
        ps = np.zeros((128, 8), np.float32)
        ps[:, c] = 1.0
        m["psel"] = ps
        in_maps.append(m)

    if _program_cache is None:
        _program_cache = _build_program()
    nc = _program_cache

    trace = bool(os.environ.get("KERNEL_TRACE"))
    tdir = os.environ.get("KERNEL_TRACE_DIR") or None
    res = run_bass_kernel_spmd(nc, in_maps, core_ids=list(range(NCORES)),
                               trace=trace, tmpdir=tdir)
    _last_results = res

    out = np.empty((N, 2 * H), np.float32)
    for c in range(NCORES):
        loc = res.results[c]["out_loc"]                  # [1024, 128]
        nloc = 128 if c == 0 else 127
        out[ids_all[c][0:nloc]] = loc[:, 0:nloc].T
    out[0:7] = res.results[0]["out_top"].T
    return out

